# revision 43
# baseline (speedup 1.0000x reference)
"""MultiHeadAttention (B=4, S=2048, D=1024, H=16, causal + key mask) on 8 trn2 cores.

Sharding: Megatron-style tensor parallel over heads. Each core owns 2 heads:
column slices of Wq/Wk/Wv (D x 128), the matching row slice of Wp (128 x D).
Each core computes a partial output y_c = attn_c @ Wp_c; host sums the 8
partials (bf16) and adds bp.

v2 redesign vs the first working kernel (551us):
  - bf16 operands everywhere (psums stay f32): halves x-read / y-write DMA,
    full PE rate at any free size (the old f32r needed N>=256).
  - Exact causal skipping: k-block (b,g,kb) computes only q in [128j, 512)
    (j = kb-4g), so exp/mask work drops to the 136-tile lower triangle, and
    the causal mask is ONE [128,128] add on the true diagonal tile only
    (the same pattern every time -> a single constant, 2-head strided op).
  - The 3.3us-per-call DVE RECIPROCAL -> reciprocal_approx_fast custom op.
  - PV psum is read directly by the normalize mul (no ACT copy); outproj
    psum->sbuf casts run on DVE, freeing ACT for exp only.
  - PE order: PV(kb) is emitted after scores(kb+3) so the PE never waits on
    the ACT exp; outproj (2 groups behind) is interleaved between score
    blocks. Keeps the PE warm (HAM K=8/8) through phase 2.
"""

import numpy as np

P = 128
B, S, D, H = 4, 2048, 1024, 16
HD = D // H  # 64
NCORES = 8
BS = B * S  # 8192
NB = S // P  # 16 k-blocks per batch
NG = S // 512  # 4 q-groups per batch

_CACHE = {}
DEBUG = False


def _build_nc():
    import concourse.mybir as mybir
    from concourse import bacc
    from concourse.tile import TileContext
    from concourse.masks import make_identity
    from contextlib import ExitStack

    f32 = mybir.dt.float32
    bf16 = mybir.dt.bfloat16
    AF = mybir.ActivationFunctionType

    nc = bacc.Bacc("TRN2", target_bir_lowering=False, debug=False,
                   num_devices=NCORES)

    xT_d = nc.dram_tensor("xT", [D, BS], bf16, kind="ExternalInput").ap()
    wq_d = nc.dram_tensor("wq", [D, P], bf16, kind="ExternalInput").ap()
    wk_d = nc.dram_tensor("wk", [D, P], bf16, kind="ExternalInput").ap()
    wv_d = nc.dram_tensor("wv", [D, P], bf16, kind="ExternalInput").ap()
    bq_d = nc.dram_tensor("bq", [P, 1], f32, kind="ExternalInput").ap()
    bk_d = nc.dram_tensor("bk", [P, 1], f32, kind="ExternalInput").ap()
    bv_d = nc.dram_tensor("bv", [P, 1], f32, kind="ExternalInput").ap()
    wp_d = nc.dram_tensor("wp", [P, D], bf16, kind="ExternalInput").ap()
    mb_d = nc.dram_tensor("maskb", [P, B * NB], f32, kind="ExternalInput").ap()
    cm_d = nc.dram_tensor("cmask", [P, 2, P], f32, kind="ExternalInput").ap()
    yp_d = nc.dram_tensor("yp", [BS, D], bf16, kind="ExternalOutput").ap()
    if DEBUG:
        dq_d = nc.dram_tensor("dq", [P, 512], bf16,
                              kind="ExternalOutput").ap()
        dk_d = nc.dram_tensor("dk", [P, 512], bf16,
                              kind="ExternalOutput").ap()
        dv_d = nc.dram_tensor("dv", [P, 2, 4, HD + 1], bf16,
                              kind="ExternalOutput").ap()
        da_d = nc.dram_tensor("da", [P, S], bf16,
                              kind="ExternalOutput").ap()
        dr_d = nc.dram_tensor("dr", [1, 2, 512], f32,
                              kind="ExternalOutput").ap()
        dp_d = nc.dram_tensor("dp", [P, 512], f32,
                              kind="ExternalOutput").ap()

    xT_r = xT_d.rearrange("(o p) n -> p o n", p=P)  # [128, 8, 8192]
    KD = D // P  # 8 contraction chunks

    with TileContext(nc) as tc:
        with ExitStack() as ctx:
            consts = ctx.enter_context(tc.tile_pool(name="consts", bufs=1))
            big = ctx.enter_context(tc.tile_pool(name="big", bufs=1))
            ptpool = ctx.enter_context(tc.tile_pool(name="ptpool", bufs=4))
            recpool = ctx.enter_context(tc.tile_pool(name="recpool", bufs=3))
            pvsbpool = ctx.enter_context(tc.tile_pool(name="pvsbpool", bufs=4))
            sxpool = ctx.enter_context(tc.tile_pool(name="sxpool", bufs=6))
            ypool = ctx.enter_context(tc.tile_pool(name="ypool", bufs=6))
            # separate pools so phase-2 scores can overlap phase-1 chunks:
            # sc2 4 banks + ps 2 + pv 2 = all 8 PSUM banks.
            pspool = ctx.enter_context(
                tc.tile_pool(name="pspool", bufs=2, space="PSUM"))
            sc2pool = ctx.enter_context(
                tc.tile_pool(name="sc2pool", bufs=2, space="PSUM"))
            pvpool = ctx.enter_context(
                tc.tile_pool(name="pvpool", bufs=2, space="PSUM"))

            # ---- constants ----
            wq_sb = consts.tile([P, KD, P], bf16, tag="wq")
            wk_sb = consts.tile([P, KD, P], bf16, tag="wk")
            wv_sb = consts.tile([P, KD, P], bf16, tag="wv")
            nc.sync.dma_start(wq_sb[:], wq_d.rearrange("(o p) m -> p o m", p=P))
            nc.sync.dma_start(wk_sb[:], wk_d.rearrange("(o p) m -> p o m", p=P))
            nc.sync.dma_start(wv_sb[:], wv_d.rearrange("(o p) m -> p o m", p=P))
            wp_sb = consts.tile([P, D], bf16, tag="wp")
            nc.sync.dma_start(wp_sb[:], wp_d)
            bq_sb = consts.tile([P, 1], f32, tag="bq")
            bk_sb = consts.tile([P, 1], f32, tag="bk")
            bv_sb = consts.tile([P, 1], f32, tag="bv")
            nc.sync.dma_start(bq_sb[:], bq_d)
            nc.sync.dma_start(bk_sb[:], bk_d)
            nc.sync.dma_start(bv_sb[:], bv_d)
            mb_sb = consts.tile([P, B * NB], f32, tag="mb")
            nc.sync.dma_start(mb_sb[:], mb_d)
            cm_sb = consts.tile([P, 2, P], f32, tag="cm")
            nc.sync.dma_start(cm_sb[:], cm_d)
            ident = consts.tile([P, P], bf16, tag="ident")
            make_identity(nc, ident[:])

            # ---- persistent activations ----
            qt_sb = big.tile([P, B, S], bf16, tag="qt")  # Q^T
            kt_sb = big.tile([P, B, S], bf16, tag="kt")  # K^T
            # V in [s, hd] layout + ones col: [p=s%128, h, b, sblock, 65]
            v_sb = big.tile([P, 2, B, NB, HD + 1], bf16, tag="v")
            nc.vector.memset(v_sb[:, :, :, :, HD:HD + 1], 1.0)
            at_sb = big.tile([P, B, S], bf16, tag="at")  # attn^T (normalized)

            # ---- phase 1: projections ----
            # The V transposes of chunk c are deferred into chunk c+1 (after
            # its Q matmuls) so the PE never waits on the ACT vt copy.
            with tc.tile_pool(name="xpool", bufs=2) as xpool, \
                 tc.tile_pool(name="vtpool", bufs=2) as vtpool:
                pend_tr = None
                for c in range(BS // 512):  # 16 chunks of 512 rows, b-major
                    b, sc = divmod(c, NG)
                    xt = xpool.tile([P, KD, 512], bf16, tag="xt")
                    nc.sync.dma_start(xt[:], xT_r[:, :, c * 512:(c + 1) * 512])
                    ssl = slice(sc * 512, (sc + 1) * 512)

                    for which in range(3):
                        w_sb = (wq_sb, wk_sb, wv_sb)[which]
                        ps = pspool.tile([P, 512], f32, tag="ps")
                        for o in range(KD):
                            nc.tensor.matmul(
                                ps[:], lhsT=w_sb[:, o, :], rhs=xt[:, o, :],
                                start=(o == 0), stop=(o == KD - 1))
                        if which == 0:
                            nc.vector.tensor_scalar_add(qt_sb[:, b, ssl],
                                                        ps[:], bq_sb[:])
                            if pend_tr is not None:
                                pend_tr()
                                pend_tr = None
                        elif which == 1:
                            nc.vector.tensor_scalar_add(kt_sb[:, b, ssl],
                                                        ps[:], bk_sb[:])
                        else:
                            vt = vtpool.tile([P, 512], bf16, tag="vt")
                            nc.scalar.activation(vt[:], ps[:], AF.Identity,
                                                 bias=bv_sb[:])

                            def tr_job(vt=vt, b=b, sc=sc, c=c):
                                trp = pspool.tile([P, 4, P], bf16, tag="ps",
                                                  name=f"trp_{c}")
                                for t in range(4):
                                    nc.tensor.transpose(
                                        trp[:, t, :],
                                        vt[:, t * P:(t + 1) * P], ident[:])
                                for h in range(2):
                                    nc.vector.tensor_copy(
                                        v_sb[:, h, b,
                                             4 * sc:4 * sc + 4, 0:HD],
                                        trp[:, :, h * HD:(h + 1) * HD])
                            pend_tr = tr_job
                if pend_tr is not None:
                    pend_tr()

            # ---- phase 2: attention + output projection ----
            cast_n = {"i": 0, "tail": False}

            def outproj_jobs(b, g):
                """Yield the 4 per-qtile emitters for group (b,g)."""
                for qc in range(4):
                    def job(qc=qc):
                        q0 = g * 512 + qc * P
                        r0 = b * S + q0
                        y_sb = ypool.tile([P, 2, 512], bf16, tag="y",
                                          name=f"y_{b}_{g}_{qc}")
                        for half in range(2):
                            # rotate yp across all three psum pools (scores
                            # and pv rings are idle in the tail) for a deep
                            # in-flight window.
                            pool, tag = ((pspool, "ps"), (sc2pool, "sc2"),
                                         (pvpool, "pv"))[cast_n["i"] % 3]
                            yp = pool.tile([P, 512], f32, tag=tag,
                                           name=f"yps_{b}_{g}_{qc}_{half}")
                            nc.tensor.matmul(
                                yp[:],
                                lhsT=at_sb[:, b, q0:q0 + P],
                                rhs=wp_sb[:, half * 512:(half + 1) * 512],
                                start=True, stop=True)
                            # outproj runs as a pure tail phase: casts go
                            # 1-in-4 to ACT, rest to DVE (Copy shares the
                            # exp activation table - no table reload).
                            cast_n["i"] += 1
                            if cast_n["i"] % 4 == 0:
                                nc.scalar.activation(y_sb[:, half, :], yp[:],
                                                     AF.Copy)
                            else:
                                nc.vector.tensor_copy(y_sb[:, half, :],
                                                      yp[:])
                        nc.gpsimd.dma_start(
                            yp_d[r0:r0 + P, :]
                            .rearrange("p (h n) -> p h n", h=2), y_sb[:])
                    yield job

            pv_pending = []   # queued PV emitters (depth 3 behind scores)
            op_pending = []   # outproj job generators, >= 3 groups behind
            stage_q = []      # stageB of the previous group

            def drain_pv(keep):
                while len(pv_pending) > keep:
                    pv_pending.pop(0)()

            def emit_outproj_some(n):
                for _ in range(n):
                    if not op_pending:
                        return
                    try:
                        job = next(op_pending[0])
                    except StopIteration:
                        op_pending.pop(0)
                        continue
                    job()

            op_ready = []  # groups whose normalize is emitted
            for b in range(B):
                for g in range(NG):
                    gsl = slice(g * 512, (g + 1) * 512)
                    nkb = 4 * (g + 1)
                    pvs = [pvpool.tile([P, 512], f32, tag="pv",
                                       name=f"pv_{b}_{g}_{h}")
                           for h in range(2)]
                    for kb in range(nkb):
                        j = kb - 4 * g
                        col = b * NB + kb
                        qo = 128 * j if j > 0 else 0
                        sc2 = sc2pool.tile([P, 2, 512], f32, tag="sc2",
                                           name=f"sc2_{b}_{g}_{kb}")
                        for h in range(2):
                            hsl = slice(h * HD, (h + 1) * HD)
                            nc.tensor.matmul(
                                sc2[:, h, qo:512],
                                lhsT=kt_sb[hsl, b, kb * P:(kb + 1) * P],
                                rhs=qt_sb[hsl, b,
                                          g * 512 + qo:(g + 1) * 512],
                                start=True, stop=True)
                        pt = ptpool.tile([P, 2, 512], bf16, tag="pt")
                        if j >= 0:  # diagonal tile: additive causal mask
                            nc.vector.tensor_add(
                                sc2[:, :, qo:qo + P], sc2[:, :, qo:qo + P],
                                cm_sb[:])
                        nc.scalar.activation(pt[:, :, qo:512],
                                             sc2[:, :, qo:512], AF.Exp,
                                             bias=mb_sb[:, col:col + 1])

                        def pv_job(kb=kb, qo=qo, pt=pt, pvs=pvs, b=b,
                                   nkb=nkb):
                            for h in range(2):
                                nc.tensor.matmul(
                                    pvs[h][0:HD + 1, qo:512],
                                    lhsT=v_sb[:, h, b, kb, :],
                                    rhs=pt[:, h, qo:512],
                                    start=(kb == 0), stop=(kb == nkb - 1))
                        pv_pending.append(pv_job)
                        drain_pv(3)

                    # Normalization is a 2-stage pipeline staggered a group
                    # apart so no in-order engine queue blocks on a fresh
                    # producer: A) free pv psum via SBUF copy + DMA-shift the
                    # denom rows to partition 0 (reciprocal_approx_fast only
                    # works at base 0); B) reciprocal + one partition-
                    # broadcast DMA + normalize muls (gpsimd) + at_sb DMA.
                    st = {}

                    def stageA(b=b, g=g, pvs=pvs, st=st):
                        pvsb = pvsbpool.tile([HD + 1, 2, 512], f32,
                                             tag="pvsb",
                                             name=f"pvsb_{b}_{g}")
                        st["pvsb"] = pvsb
                        for h in range(2):
                            nc.vector.tensor_copy(pvsb[:, h, :],
                                                  pvs[h][0:HD + 1, :])
                        st["dn"] = recpool.tile([1, 2, 512], f32, tag="dn",
                                                name=f"dn_{b}_{g}")
                        nc.sync.dma_start(st["dn"][:],
                                          pvsb[HD:HD + 1, :, :])

                    def stageB(b=b, g=g, gsl=gsl, st=st):
                        pvsb = st["pvsb"]
                        rec = recpool.tile([1, 2, 512], f32, tag="rec",
                                           name=f"rec_{b}_{g}")
                        nc.vector.reciprocal_approx_fast(
                            rec[:, :, :], st["dn"][:, :, :])
                        if DEBUG and b == 0 and g == 0:
                            nc.sync.dma_start(dp_d[0:HD + 1, :],
                                              pvsb[:, 0, :])
                            nc.sync.dma_start(dr_d[:], rec[:])
                        sx = sxpool.tile([HD, 2, 512], f32, tag="sx",
                                         name=f"sx_{b}_{g}")
                        nc.sync.dma_start(
                            sx[:],
                            rec[:, None, :, :]
                            .to_broadcast((1, HD, 2, 512)))
                        nc.gpsimd.tensor_mul(
                            at_sb[0:HD, b, gsl],
                            pvsb[0:HD, 0, :], sx[:, 0, :])
                        # head 1 lands on partitions 64..127: engines cannot
                        # shift partitions -> mul to a tmp, DMA into place
                        # (same gpsimd queue as the mul: no cross-queue block)
                        tmp = sxpool.tile([HD, 512], bf16, tag="tmp",
                                          name=f"tmp_{b}_{g}")
                        nc.gpsimd.tensor_mul(
                            tmp[:], pvsb[0:HD, 1, :], sx[:, 1, :])
                        nc.gpsimd.dma_start(at_sb[HD:2 * HD, b, gsl], tmp[:])

                    # stage A goes through pv_pending (must follow the last
                    # PV of this group); B runs at the next group end.
                    pv_pending.append(stageA)
                    if stage_q:
                        stage_q.pop(0)()         # B of group i-1
                    stage_q.append(stageB)
                    op_ready.append((b, g))

            drain_pv(0)
            while stage_q:
                stage_q.pop(0)()
            cast_n["tail"] = True
            for bg in op_ready:
                op_pending.append(outproj_jobs(*bg))
            emit_outproj_some(1000)

            if DEBUG:
                nc.sync.dma_start(dq_d[:], qt_sb[:, 0, 0:512])
                nc.sync.dma_start(dk_d[:], kt_sb[:, 0, 0:512])
                nc.sync.dma_start(dv_d[:], v_sb[:, :, 0, 0:4, :])
                nc.sync.dma_start(da_d[:], at_sb[:, 0, :])

    nc.compile()
    return nc


def _get_nc():
    if "nc" not in _CACHE:
        _CACHE["nc"] = _build_nc()
    return _CACHE["nc"]


def make_in_maps(x, attention_mask, Wq, bq, Wk, bk, Wv, bv, Wp, bp):
    """Host-side sharding: build the 8 per-core device input maps."""
    import ml_dtypes
    bf16 = ml_dtypes.bfloat16
    x = np.asarray(x, dtype=np.float32)
    scale = np.float32(1.0 / np.sqrt(HD))
    xT = np.ascontiguousarray(x.reshape(BS, D).T.astype(bf16))  # [D, BS]
    mb = (np.asarray(attention_mask).astype(np.float32) - 1.0) * np.float32(1e9)
    mb = np.ascontiguousarray(
        mb.reshape(B, NB, P).transpose(2, 0, 1).reshape(P, B * NB))
    # causal diagonal-tile mask (additive): 0 where q_local >= k_local,
    # else -1e9; duplicated for the two heads' strided slices.
    pp = np.arange(P)[:, None]
    ff = np.arange(P)[None, :]
    cm1 = np.where(ff >= pp, 0.0, -1e9).astype(np.float32)
    cm = np.ascontiguousarray(
        np.stack([cm1, cm1], axis=1))  # [128, 2, 128]

    Wq = (np.asarray(Wq, np.float32) * scale).astype(bf16)
    bq = np.asarray(bq, np.float32) * scale
    Wk = np.asarray(Wk, np.float32).astype(bf16)
    bk = np.asarray(bk, np.float32)
    Wv = np.asarray(Wv, np.float32).astype(bf16)
    bv = np.asarray(bv, np.float32)
    Wp = np.asarray(Wp, np.float32).astype(bf16)

    in_maps = []
    for c in range(NCORES):
        cs = slice(c * P, (c + 1) * P)
        in_maps.append({
            "xT": xT,
            "wq": np.ascontiguousarray(Wq[:, cs]),
            "wk": np.ascontiguousarray(Wk[:, cs]),
            "wv": np.ascontiguousarray(Wv[:, cs]),
            "bq": np.ascontiguousarray(bq[cs].reshape(P, 1)),
            "bk": np.ascontiguousarray(bk[cs].reshape(P, 1)),
            "bv": np.ascontiguousarray(bv[cs].reshape(P, 1)),
            "wp": np.ascontiguousarray(Wp[cs, :]),
            "maskb": mb,
            "cmask": cm,
        })
    return in_maps


def run(inputs, trace=False, tmpdir=None):
    """Compile (cached) + run on 8 cores. Returns (output, BassKernelResults)."""
    from concourse import bass_utils
    nc = _get_nc()
    in_maps = make_in_maps(**inputs)
    kwargs = {}
    if trace:
        kwargs = dict(trace=True, tmpdir=tmpdir)
    res = bass_utils.run_bass_kernel_spmd(
        nc, in_maps, core_ids=list(range(NCORES)), **kwargs)
    acc = np.zeros((BS, D), dtype=np.float64)
    for r in res.results:
        acc += r["yp"].astype(np.float64)
    out = (acc + np.asarray(inputs["bp"], np.float64)[None, :]).astype(
        np.float32)
    return out.reshape(B, S, D), res


def kernel(**inputs) -> np.ndarray:
    out, _ = run(inputs, trace=False)
    return out


# revision 50
# speedup vs baseline: 1.0126x; 1.0126x over previous
"""MultiHeadAttention (B=4, S=2048, D=1024, H=16, causal + key mask) on 8 trn2 cores.

Sharding: Megatron-style tensor parallel over heads. Each core owns 2 heads:
column slices of Wq/Wk/Wv (D x 128), the matching row slice of Wp (128 x D).
Each core computes a partial output y_c = attn_c @ Wp_c; host sums the 8
partials (bf16) and adds bp.

v2 redesign vs the first working kernel (551us):
  - bf16 operands everywhere (psums stay f32): halves x-read / y-write DMA,
    full PE rate at any free size (the old f32r needed N>=256).
  - Exact causal skipping: k-block (b,g,kb) computes only q in [128j, 512)
    (j = kb-4g), so exp/mask work drops to the 136-tile lower triangle, and
    the causal mask is ONE [128,128] add on the true diagonal tile only
    (the same pattern every time -> a single constant, 2-head strided op).
  - The 3.3us-per-call DVE RECIPROCAL -> reciprocal_approx_fast custom op.
  - PV psum is read directly by the normalize mul (no ACT copy); outproj
    psum->sbuf casts run on DVE, freeing ACT for exp only.
  - PE order: PV(kb) is emitted after scores(kb+3) so the PE never waits on
    the ACT exp; outproj (2 groups behind) is interleaved between score
    blocks. Keeps the PE warm (HAM K=8/8) through phase 2.
"""

import numpy as np

P = 128
B, S, D, H = 4, 2048, 1024, 16
HD = D // H  # 64
NCORES = 8
BS = B * S  # 8192
NB = S // P  # 16 k-blocks per batch
NG = S // 512  # 4 q-groups per batch

_CACHE = {}
DEBUG = False


def _build_nc():
    import concourse.mybir as mybir
    from concourse import bacc
    from concourse.tile import TileContext
    from concourse.masks import make_identity
    from contextlib import ExitStack

    f32 = mybir.dt.float32
    bf16 = mybir.dt.bfloat16
    AF = mybir.ActivationFunctionType

    nc = bacc.Bacc("TRN2", target_bir_lowering=False, debug=False,
                   num_devices=NCORES)

    xT_d = nc.dram_tensor("xT", [D, BS], bf16, kind="ExternalInput").ap()
    wq_d = nc.dram_tensor("wq", [D, P], bf16, kind="ExternalInput").ap()
    wk_d = nc.dram_tensor("wk", [D, P], bf16, kind="ExternalInput").ap()
    wv_d = nc.dram_tensor("wv", [D, P], bf16, kind="ExternalInput").ap()
    bq_d = nc.dram_tensor("bq", [P, 1], f32, kind="ExternalInput").ap()
    bk_d = nc.dram_tensor("bk", [P, 1], f32, kind="ExternalInput").ap()
    bv_d = nc.dram_tensor("bv", [P, 1], f32, kind="ExternalInput").ap()
    wp_d = nc.dram_tensor("wp", [P, D], bf16, kind="ExternalInput").ap()
    mb_d = nc.dram_tensor("maskb", [P, B * NB], f32, kind="ExternalInput").ap()
    cm_d = nc.dram_tensor("cmask", [P, 2, P], f32, kind="ExternalInput").ap()
    yp_d = nc.dram_tensor("yp", [BS, D], bf16, kind="ExternalOutput").ap()
    if DEBUG:
        dq_d = nc.dram_tensor("dq", [P, 512], bf16,
                              kind="ExternalOutput").ap()
        dk_d = nc.dram_tensor("dk", [P, 512], bf16,
                              kind="ExternalOutput").ap()
        dv_d = nc.dram_tensor("dv", [P, 2, 4, HD + 1], bf16,
                              kind="ExternalOutput").ap()
        da_d = nc.dram_tensor("da", [P, S], bf16,
                              kind="ExternalOutput").ap()
        dr_d = nc.dram_tensor("dr", [1, 2, 512], f32,
                              kind="ExternalOutput").ap()
        dp_d = nc.dram_tensor("dp", [P, 512], f32,
                              kind="ExternalOutput").ap()

    xT_r = xT_d.rearrange("(o p) n -> p o n", p=P)  # [128, 8, 8192]
    KD = D // P  # 8 contraction chunks

    with TileContext(nc) as tc:
        with ExitStack() as ctx:
            consts = ctx.enter_context(tc.tile_pool(name="consts", bufs=1))
            big = ctx.enter_context(tc.tile_pool(name="big", bufs=1))
            ptpool = ctx.enter_context(tc.tile_pool(name="ptpool", bufs=4))
            recpool = ctx.enter_context(tc.tile_pool(name="recpool", bufs=3))
            pvsbpool = ctx.enter_context(tc.tile_pool(name="pvsbpool", bufs=4))
            sxpool = ctx.enter_context(tc.tile_pool(name="sxpool", bufs=6))
            ypool = ctx.enter_context(tc.tile_pool(name="ypool", bufs=6))
            # separate pools so phase-2 scores can overlap phase-1 chunks:
            # sc2 4 banks + ps 2 + pv 2 = all 8 PSUM banks.
            pspool = ctx.enter_context(
                tc.tile_pool(name="pspool", bufs=2, space="PSUM"))
            sc2pool = ctx.enter_context(
                tc.tile_pool(name="sc2pool", bufs=2, space="PSUM"))
            pvpool = ctx.enter_context(
                tc.tile_pool(name="pvpool", bufs=2, space="PSUM"))

            # ---- constants (gpsimd DMA queue: keeps the sync queue free so
            # the first x chunk streams immediately) ----
            wq_sb = consts.tile([P, KD, P], bf16, tag="wq")
            wk_sb = consts.tile([P, KD, P], bf16, tag="wk")
            wv_sb = consts.tile([P, KD, P], bf16, tag="wv")
            nc.gpsimd.dma_start(wq_sb[:],
                                wq_d.rearrange("(o p) m -> p o m", p=P))
            nc.gpsimd.dma_start(wk_sb[:],
                                wk_d.rearrange("(o p) m -> p o m", p=P))
            nc.gpsimd.dma_start(wv_sb[:],
                                wv_d.rearrange("(o p) m -> p o m", p=P))
            wp_sb = consts.tile([P, D], bf16, tag="wp")
            nc.gpsimd.dma_start(wp_sb[:], wp_d)
            bq_sb = consts.tile([P, 1], f32, tag="bq")
            bk_sb = consts.tile([P, 1], f32, tag="bk")
            bv_sb = consts.tile([P, 1], f32, tag="bv")
            nc.gpsimd.dma_start(bq_sb[:], bq_d)
            nc.gpsimd.dma_start(bk_sb[:], bk_d)
            nc.gpsimd.dma_start(bv_sb[:], bv_d)
            mb_sb = consts.tile([P, B * NB], f32, tag="mb")
            nc.gpsimd.dma_start(mb_sb[:], mb_d)
            cm_sb = consts.tile([P, 2, P], f32, tag="cm")
            nc.gpsimd.dma_start(cm_sb[:], cm_d)
            ident = consts.tile([P, P], bf16, tag="ident")
            make_identity(nc, ident[:])

            # ---- persistent activations ----
            qt_sb = big.tile([P, B, S], bf16, tag="qt")  # Q^T
            kt_sb = big.tile([P, B, S], bf16, tag="kt")  # K^T
            # V in [s, hd] layout + ones col: [p=s%128, h, b, sblock, 65]
            v_sb = big.tile([P, 2, B, NB, HD + 1], bf16, tag="v")
            nc.vector.memset(v_sb[:, :, :, :, HD:HD + 1], 1.0)
            at_sb = big.tile([P, B, S], bf16, tag="at")  # attn^T (normalized)

            # ---- phase 1: projections ----
            # The V transposes of chunk c are deferred into chunk c+1 (after
            # its Q matmuls) so the PE never waits on the ACT vt copy.
            with tc.tile_pool(name="xpool", bufs=2) as xpool, \
                 tc.tile_pool(name="vtpool", bufs=2) as vtpool:
                pend_tr = None
                for c in range(BS // 512):  # 16 chunks of 512 rows, b-major
                    b, sc = divmod(c, NG)
                    xt = xpool.tile([P, KD, 512], bf16, tag="xt")
                    nc.sync.dma_start(xt[:], xT_r[:, :, c * 512:(c + 1) * 512])
                    ssl = slice(sc * 512, (sc + 1) * 512)

                    for which in range(3):
                        w_sb = (wq_sb, wk_sb, wv_sb)[which]
                        ps = pspool.tile([P, 512], f32, tag="ps")
                        for o in range(KD):
                            nc.tensor.matmul(
                                ps[:], lhsT=w_sb[:, o, :], rhs=xt[:, o, :],
                                start=(o == 0), stop=(o == KD - 1))
                        if which == 0:
                            nc.vector.tensor_scalar_add(qt_sb[:, b, ssl],
                                                        ps[:], bq_sb[:])
                            if pend_tr is not None:
                                pend_tr()
                                pend_tr = None
                        elif which == 1:
                            nc.vector.tensor_scalar_add(kt_sb[:, b, ssl],
                                                        ps[:], bk_sb[:])
                        else:
                            vt = vtpool.tile([P, 512], bf16, tag="vt")
                            nc.scalar.activation(vt[:], ps[:], AF.Identity,
                                                 bias=bv_sb[:])

                            def tr_job(vt=vt, b=b, sc=sc, c=c):
                                trp = pspool.tile([P, 4, P], bf16, tag="ps",
                                                  name=f"trp_{c}")
                                for t in range(4):
                                    nc.tensor.transpose(
                                        trp[:, t, :],
                                        vt[:, t * P:(t + 1) * P], ident[:])
                                for h in range(2):
                                    nc.vector.tensor_copy(
                                        v_sb[:, h, b,
                                             4 * sc:4 * sc + 4, 0:HD],
                                        trp[:, :, h * HD:(h + 1) * HD])
                            pend_tr = tr_job
                if pend_tr is not None:
                    pend_tr()

            # ---- phase 2: attention + output projection ----
            cast_n = {"i": 0, "tail": False}

            def outproj_jobs(b, g):
                """Yield the 4 per-qtile emitters for group (b,g)."""
                for qc in range(4):
                    def job(qc=qc):
                        q0 = g * 512 + qc * P
                        r0 = b * S + q0
                        y_sb = ypool.tile([P, 2, 512], bf16, tag="y",
                                          name=f"y_{b}_{g}_{qc}")
                        for half in range(2):
                            # rotate yp across all three psum pools (scores
                            # and pv rings are idle in the tail) for a deep
                            # in-flight window.
                            pool, tag = ((pspool, "ps"), (sc2pool, "sc2"),
                                         (pvpool, "pv"))[cast_n["i"] % 3]
                            yp = pool.tile([P, 512], f32, tag=tag,
                                           name=f"yps_{b}_{g}_{qc}_{half}")
                            nc.tensor.matmul(
                                yp[:],
                                lhsT=at_sb[:, b, q0:q0 + P],
                                rhs=wp_sb[:, half * 512:(half + 1) * 512],
                                start=True, stop=True)
                            # casts alternate engines 50/50 in the tail
                            # (both idle there; Copy shares the exp act
                            # table), mostly-DVE while the main loop runs.
                            cast_n["i"] += 1
                            if cast_n["tail"] and cast_n["i"] % 2 == 0:
                                nc.scalar.activation(y_sb[:, half, :], yp[:],
                                                     AF.Copy)
                            else:
                                nc.vector.tensor_copy(y_sb[:, half, :],
                                                      yp[:])
                        nc.gpsimd.dma_start(
                            yp_d[r0:r0 + P, :]
                            .rearrange("p (h n) -> p h n", h=2), y_sb[:])
                    yield job

            pv_pending = []   # queued PV emitters (depth 3 behind scores)
            op_pending = []   # outproj job generators, >= 3 groups behind
            stage_q = []      # stageB of the previous group

            def drain_pv(keep):
                while len(pv_pending) > keep:
                    pv_pending.pop(0)()

            def emit_outproj_some(n):
                for _ in range(n):
                    if not op_pending:
                        return
                    try:
                        job = next(op_pending[0])
                    except StopIteration:
                        op_pending.pop(0)
                        continue
                    job()

            op_ready = []  # groups whose normalize is emitted
            for b in range(B):
                for g in range(NG):
                    gsl = slice(g * 512, (g + 1) * 512)
                    nkb = 4 * (g + 1)
                    pvs = [pvpool.tile([P, 512], f32, tag="pv",
                                       name=f"pv_{b}_{g}_{h}")
                           for h in range(2)]
                    for kb in range(nkb):
                        j = kb - 4 * g
                        col = b * NB + kb
                        qo = 128 * j if j > 0 else 0
                        sc2 = sc2pool.tile([P, 2, 512], f32, tag="sc2",
                                           name=f"sc2_{b}_{g}_{kb}")
                        for h in range(2):
                            hsl = slice(h * HD, (h + 1) * HD)
                            nc.tensor.matmul(
                                sc2[:, h, qo:512],
                                lhsT=kt_sb[hsl, b, kb * P:(kb + 1) * P],
                                rhs=qt_sb[hsl, b,
                                          g * 512 + qo:(g + 1) * 512],
                                start=True, stop=True)
                        pt = ptpool.tile([P, 2, 512], bf16, tag="pt")
                        if j >= 0:  # diagonal tile: additive causal mask
                            nc.vector.tensor_add(
                                sc2[:, :, qo:qo + P], sc2[:, :, qo:qo + P],
                                cm_sb[:])
                        nc.scalar.activation(pt[:, :, qo:512],
                                             sc2[:, :, qo:512], AF.Exp,
                                             bias=mb_sb[:, col:col + 1])

                        def pv_job(kb=kb, qo=qo, pt=pt, pvs=pvs, b=b,
                                   nkb=nkb):
                            for h in range(2):
                                nc.tensor.matmul(
                                    pvs[h][0:HD + 1, qo:512],
                                    lhsT=v_sb[:, h, b, kb, :],
                                    rhs=pt[:, h, qo:512],
                                    start=(kb == 0), stop=(kb == nkb - 1))
                        pv_pending.append(pv_job)
                        drain_pv(3)

                    # Normalization is a 2-stage pipeline staggered a group
                    # apart so no in-order engine queue blocks on a fresh
                    # producer: A) free pv psum via SBUF copy + DMA-shift the
                    # denom rows to partition 0 (reciprocal_approx_fast only
                    # works at base 0); B) reciprocal + one partition-
                    # broadcast DMA + normalize muls (gpsimd) + at_sb DMA.
                    st = {}

                    def stageA(b=b, g=g, pvs=pvs, st=st):
                        pvsb = pvsbpool.tile([HD + 1, 2, 512], f32,
                                             tag="pvsb",
                                             name=f"pvsb_{b}_{g}")
                        st["pvsb"] = pvsb
                        for h in range(2):
                            nc.vector.tensor_copy(pvsb[:, h, :],
                                                  pvs[h][0:HD + 1, :])
                        st["dn"] = recpool.tile([1, 2, 512], f32, tag="dn",
                                                name=f"dn_{b}_{g}")
                        nc.sync.dma_start(st["dn"][:],
                                          pvsb[HD:HD + 1, :, :])

                    def stageB(b=b, g=g, gsl=gsl, st=st):
                        pvsb = st["pvsb"]
                        rec = recpool.tile([1, 2, 512], f32, tag="rec",
                                           name=f"rec_{b}_{g}")
                        nc.vector.reciprocal_approx_fast(
                            rec[:, :, :], st["dn"][:, :, :])
                        if DEBUG and b == 0 and g == 0:
                            nc.sync.dma_start(dp_d[0:HD + 1, :],
                                              pvsb[:, 0, :])
                            nc.sync.dma_start(dr_d[:], rec[:])
                        sx = sxpool.tile([HD, 2, 512], f32, tag="sx",
                                         name=f"sx_{b}_{g}")
                        nc.sync.dma_start(
                            sx[:],
                            rec[:, None, :, :]
                            .to_broadcast((1, HD, 2, 512)))
                        nc.gpsimd.tensor_mul(
                            at_sb[0:HD, b, gsl],
                            pvsb[0:HD, 0, :], sx[:, 0, :])
                        # head 1 lands on partitions 64..127: engines cannot
                        # shift partitions -> mul to a tmp, DMA into place
                        # (same gpsimd queue as the mul: no cross-queue block)
                        tmp = sxpool.tile([HD, 512], bf16, tag="tmp",
                                          name=f"tmp_{b}_{g}")
                        nc.gpsimd.tensor_mul(
                            tmp[:], pvsb[0:HD, 1, :], sx[:, 1, :])
                        nc.gpsimd.dma_start(at_sb[HD:2 * HD, b, gsl], tmp[:])

                    # stage A goes through pv_pending (must follow the last
                    # PV of this group); B runs at the next group end.
                    pv_pending.append(stageA)
                    if stage_q:
                        stage_q.pop(0)()         # B of group i-1
                    stage_q.append(stageB)
                    op_ready.append((b, g))

            drain_pv(0)
            while stage_q:
                stage_q.pop(0)()
            cast_n["tail"] = True
            for bg in op_ready:
                op_pending.append(outproj_jobs(*bg))
            emit_outproj_some(1000)

            if DEBUG:
                nc.sync.dma_start(dq_d[:], qt_sb[:, 0, 0:512])
                nc.sync.dma_start(dk_d[:], kt_sb[:, 0, 0:512])
                nc.sync.dma_start(dv_d[:], v_sb[:, :, 0, 0:4, :])
                nc.sync.dma_start(da_d[:], at_sb[:, 0, :])

    nc.compile()
    return nc


def _get_nc():
    if "nc" not in _CACHE:
        _CACHE["nc"] = _build_nc()
    return _CACHE["nc"]


def make_in_maps(x, attention_mask, Wq, bq, Wk, bk, Wv, bv, Wp, bp):
    """Host-side sharding: build the 8 per-core device input maps."""
    import ml_dtypes
    bf16 = ml_dtypes.bfloat16
    x = np.asarray(x, dtype=np.float32)
    scale = np.float32(1.0 / np.sqrt(HD))
    xT = np.ascontiguousarray(x.reshape(BS, D).T.astype(bf16))  # [D, BS]
    mb = (np.asarray(attention_mask).astype(np.float32) - 1.0) * np.float32(1e9)
    mb = np.ascontiguousarray(
        mb.reshape(B, NB, P).transpose(2, 0, 1).reshape(P, B * NB))
    # causal diagonal-tile mask (additive): 0 where q_local >= k_local,
    # else -1e9; duplicated for the two heads' strided slices.
    pp = np.arange(P)[:, None]
    ff = np.arange(P)[None, :]
    cm1 = np.where(ff >= pp, 0.0, -1e9).astype(np.float32)
    cm = np.ascontiguousarray(
        np.stack([cm1, cm1], axis=1))  # [128, 2, 128]

    Wq = (np.asarray(Wq, np.float32) * scale).astype(bf16)
    bq = np.asarray(bq, np.float32) * scale
    Wk = np.asarray(Wk, np.float32).astype(bf16)
    bk = np.asarray(bk, np.float32)
    Wv = np.asarray(Wv, np.float32).astype(bf16)
    bv = np.asarray(bv, np.float32)
    Wp = np.asarray(Wp, np.float32).astype(bf16)

    in_maps = []
    for c in range(NCORES):
        cs = slice(c * P, (c + 1) * P)
        in_maps.append({
            "xT": xT,
            "wq": np.ascontiguousarray(Wq[:, cs]),
            "wk": np.ascontiguousarray(Wk[:, cs]),
            "wv": np.ascontiguousarray(Wv[:, cs]),
            "bq": np.ascontiguousarray(bq[cs].reshape(P, 1)),
            "bk": np.ascontiguousarray(bk[cs].reshape(P, 1)),
            "bv": np.ascontiguousarray(bv[cs].reshape(P, 1)),
            "wp": np.ascontiguousarray(Wp[cs, :]),
            "maskb": mb,
            "cmask": cm,
        })
    return in_maps


def run(inputs, trace=False, tmpdir=None):
    """Compile (cached) + run on 8 cores. Returns (output, BassKernelResults)."""
    from concourse import bass_utils
    nc = _get_nc()
    in_maps = make_in_maps(**inputs)
    kwargs = {}
    if trace:
        kwargs = dict(trace=True, tmpdir=tmpdir)
    res = bass_utils.run_bass_kernel_spmd(
        nc, in_maps, core_ids=list(range(NCORES)), **kwargs)
    acc = np.zeros((BS, D), dtype=np.float64)
    for r in res.results:
        acc += r["yp"].astype(np.float64)
    out = (acc + np.asarray(inputs["bp"], np.float64)[None, :]).astype(
        np.float32)
    return out.reshape(B, S, D), res


def kernel(**inputs) -> np.ndarray:
    out, _ = run(inputs, trace=False)
    return out
